# revision 32
# baseline (speedup 1.0000x reference)
"""Trainium2 Bass kernel for nn_BiquadCoeffFilter_31628139167986.

Reference computation (per batch row, T = 262144 samples):
  logits = linear-interp of 256 control points -> T samples (5 channels)
  a1 = 2*tanh(l0)*stab ; a2 = 0.5*((2-|a1|)*tanh(l1)*stab + |a1|)  (stab = 1-1e-3)
  IIR:  y[t] = x[t] - a1[t]*y[t-1] - a2[t]*y[t-2]
  FIR:  out[t] = b0[t]*y[t] + b1[t]*y[t-1] + b2[t]*y[t-2],  b = logits[..., 2:5]

Sharding: pure data parallel, 4 batch rows per core x 8 NeuronCores (SPMD).

Per-core pipeline:
  A. Coefficient generation in SEGMENT layout (partition = one interpolation
     segment window of 1032 samples; the interpolated logit is affine in the
     in-window position, so ScalarE computes tanh(w*d + v0) with per-partition
     scale/bias, fusing interpolation into the activation).  ALL five
     per-sample coefficient planes (na1 = -a1, na2 = -a2 in fp32; b0..b2 in
     bf16) are scattered to time-linear DRAM stages in one pass.
  B. Chunked 3-solution scan in SCAN layout [128 partitions = 8192-sample
     stretches, 256 chunks x 32 steps]: per chunk the zero-state response
     y_zero (DVE+GPSIMD split) and the two homogeneous responses h1 (DVE) /
     h2 (GPSIMD) are computed in 32 vectorized steps.
  C. Per-chunk affine transfer maps (2x2 M, offset p) are prefix-composed
     hierarchically; correction y = y_zero + alpha*h1 + beta*h2 with
     stride-0 broadcast of the per-chunk entry states.  The b-plane gathers
     run concurrently on otherwise-idle queues.
  D. FIR directly in scan layout (bf16 products; 2-sample cross-partition
     halo), final fp32 cast on ScalarE, one linear DMA to the output.
"""
import sys
sys.path.insert(0, '/opt/trn_rl_repo')
import numpy as np

B, T = 32, 262144
NSEG = 255
SEGLEN = 87381      # (T-1)/3 ; 3 super-blocks x 85 segments per row
SUP = 85
ROWS = 4
NCORES = 8
L1 = 32             # chunk length
NSTR = 32           # stretches per row
STR = T // NSTR     # 8192
CPS = STR // L1     # 256 chunks per stretch
WIN = 1032
PAD = 4
TP = T + 2 * PAD
DELTA = float(NSEG) / float(T - 1)
STAB = 1.0 - 1e-3

_PATCHED = False


def _patch_tile_drain():
    """This toolchain allows a single sem wait per instruction; split the tile
    tail-drain's accumulated waits across chained drain instructions."""
    global _PATCHED
    if _PATCHED:
        return
    from concourse import tile, mybir
    from concourse.vector_clock import ScopedClock

    def _drain_and_barrier_split(self, tick_clock, wait_clock):
        drain_inst = self.nc.sync.drain()
        wait_clock.add_sem_waits(
            drain_inst.ins, ScopedClock({None: tick_clock.global_clock}))
        si = drain_inst.ins.sync_info
        waits = list(si.on_wait or []) if si else []
        if len(waits) > 1:
            si.on_wait = waits[:1]
            for i in range(1, len(waits)):
                d2 = self.nc.sync.drain()
                d2.ins.sync_info = mybir.SyncInfo(on_wait=[waits[i]], on_update=[])
        self.nc.all_engine_barrier()
        assert self.sems is not None
        popped = self.nc._tile_sem_poison_stack.pop()
        assert popped is self._sem_poison
        self.nc.clear_and_free_semaphores(list(self.sems.allocated().values()))
        self.nc.all_engine_barrier()

    tile.TileContext._drain_and_barrier = _drain_and_barrier_split
    _PATCHED = True


def _fix_multi_waits(nc):
    """Hoist extra sem waits onto same-engine nops (1-wait codegen limit)."""
    from concourse import mybir

    def make_nop(engine):
        bi = nc.engines[engine].nop(nofuse=True, hint="wait_split")
        inst = bi.ins
        for f in nc.m.functions:
            for bb in f.blocks:
                il = bb.instructions
                if il and il[-1] is inst:
                    bb.instructions = il[:-1]
                    return inst
        raise RuntimeError("nop not found")

    for f in nc.m.functions:
        for bb in f.blocks:
            il = list(bb.instructions)
            out = []
            changed = False
            for inst in il:
                si = getattr(inst, 'sync_info', None)
                waits = list(si.on_wait or []) if si else []
                if len(waits) > 1 and getattr(inst, 'engine', None) is not None:
                    changed = True
                    extra, keep = waits[:-1], waits[-1:]
                    for w in extra:
                        nop = make_nop(inst.engine)
                        nop.sync_info = mybir.SyncInfo(on_wait=[w], on_update=[])
                        out.append(nop)
                    si.on_wait = keep
                out.append(inst)
            if changed:
                bb.instructions = out


def _lane_runs():
    """lane = r*255 + 85*k + sp  (row r, super-block k, segment sp).
    Runs of consecutive sp split at 128-partition tile boundaries.
    Returns (tile, part0, r, k, sp0, n)."""
    runs = []
    for r in range(ROWS):
        for k in range(3):
            base = r * NSEG + SUP * k
            sp = 0
            while sp < SUP:
                lane = base + sp
                tile_i, part = divmod(lane, 128)
                n = min(SUP - sp, 128 - part)
                runs.append((tile_i, part, r, k, sp, n))
                sp += n
    return runs


RUNS = _lane_runs()


def host_tables():
    w0 = np.zeros((128, 8, 1), np.float32)
    for r in range(ROWS):
        for k in range(3):
            for sp in range(SUP):
                lane = r * NSEG + SUP * k + sp
                seg = SUP * k + sp
                wstart = SEGLEN * k + 1028 * sp - 2
                w0[lane % 128, lane // 128, 0] = np.float64(wstart) * DELTA - seg
    iota = np.arange(WIN, dtype=np.float32)[None, :].repeat(128, 0)
    return w0, iota


def host_v0v1(cl_rows):
    """Per-lane control-point values [8,128,5] (pure data movement)."""
    v0 = np.zeros((128, 8, 5), np.float32)
    v1 = np.zeros((128, 8, 5), np.float32)
    for r in range(ROWS):
        for seg in range(NSEG):
            lane = r * NSEG + seg
            v0[lane % 128, lane // 128] = cl_rows[r, seg]
            v1[lane % 128, lane // 128] = cl_rows[r, seg + 1]
    return v0, v1


def build_program(loop_n=None):
    from contextlib import nullcontext
    from concourse import bass, mybir
    from concourse.tile import TileContext
    fp32 = mybir.dt.float32
    bf16 = mybir.dt.bfloat16
    Alu = mybir.AluOpType
    Act = mybir.ActivationFunctionType

    nc = bass.Bass("TRN2", target_bir_lowering=False, debug=False)
    pe_q = nc.engines[mybir.EngineType.PE]

    x_in = nc.dram_tensor("x", [ROWS, T], fp32, kind="ExternalInput").ap()
    v0_in = nc.dram_tensor("v0", [128, 8, 5], fp32, kind="ExternalInput").ap()
    v1_in = nc.dram_tensor("v1", [128, 8, 5], fp32, kind="ExternalInput").ap()
    w0_in = nc.dram_tensor("w0", [128, 8, 1], fp32, kind="ExternalInput").ap()
    iota_in = nc.dram_tensor("iota", [128, WIN], fp32, kind="ExternalInput").ap()
    last8_in = nc.dram_tensor("last8", [ROWS, 5], fp32,
                              kind="ExternalInput").ap()
    firstk_in = nc.dram_tensor("firstk", [ROWS, 3, 5], fp32,
                               kind="ExternalInput").ap()
    y_out = nc.dram_tensor("y", [ROWS, T], fp32, kind="ExternalOutput").ap()

    st_na = nc.dram_tensor("st_na", [ROWS, 2, TP], fp32).ap()
    st_b = nc.dram_tensor("st_b", [ROWS, 3, TP], bf16).ap()
    st_cmp = nc.dram_tensor("st_cmp", [128, 6], fp32).ap()
    st_sin = nc.dram_tensor("st_sin", [ROWS, NSTR, 2], fp32).ap()

    import bass_rust

    # DMA issuance is phase-aware: per-call engine choice keeps the holds on
    # queues that are otherwise idle in that phase.  gpsimd (SWDGE) DMAs are
    # remapped onto the two HWDGE queues: a software-DGE DMA inside a
    # hardware loop trips an "ISA wrong length" walrus codegen bug, and the
    # timed (For_i-wrapped) program must be instruction-identical to this
    # one.
    _gp_alt = [0]

    def dma(out, in_, eng):
        if eng is nc.gpsimd:
            _gp_alt[0] += 1
            eng = nc.sync if _gp_alt[0] % 2 else nc.scalar
        return eng.dma_start(out=out, in_=in_)

    def na_dst(r, start, n, ln):
        # [n segs (stride 1028), 2 planes, ln cols] into row-r na stage
        v = st_na[r, 0, :].copy()
        v.ap = bass_rust.VecI64Pair([[1028, n], [TP, 2], [1, ln]])
        v.offset = v.offset + start
        return v

    def b_dst(r, start, n, ln):
        v = st_b[r, 0, :].copy()
        v.ap = bass_rust.VecI64Pair([[1028, n], [TP, 3], [1, ln]])
        v.offset = v.offset + start
        return v

    def scatter_plane(ti, tap, dstf, nplane, eng_sel):
        """Scatter all planes of window tile ti ([128, nplane, WIN]) to the
        time-linear stage via dstf(r, start, n, ln)."""
        # every run is uniform-stride: super-block start samples (exactly on
        # knots) come from the host-exact firstk input instead
        for (tj, part, r, k, sp0, n) in [u for u in RUNS if u[0] == ti]:
            dma(out=dstf(r, PAD + SEGLEN * k + 1028 * sp0 + 1, n, 1028),
                in_=tap[part: part + n, :, 3:WIN - 1], eng=eng_sel(0))
        # sample T-1 is patched from the host-exact last8 input instead

    # affine map composition: prefix along last axis of [P, nblk, L] comps
    def prefix_chain(comps, tmps, P, nblk, L, gp_tmps=None):
        """In-place inclusive prefix of 2x2 affine maps along the last axis.
        Row-1 (m11/m12/q1) runs on DVE, row-2 (m21/m22/q2) on GPSIMD."""
        (m11, m12, m21, m22, q1, q2) = comps
        (t1, t2, t3, t4, t5, t6) = tmps
        if gp_tmps is None:
            gp_tmps = tmps
        (g1, g2, g3, g4, g5, g6) = gp_tmps
        for k in range(1, L):
            cur = lambda a: a[:, :, k]
            prv = lambda a: a[:, :, k - 1]
            nc.vector.tensor_tensor(t1, cur(m11), prv(m11), Alu.mult)
            nc.vector.tensor_tensor(t2, cur(m11), prv(m12), Alu.mult)
            nc.vector.tensor_tensor(t3, cur(m11), prv(q1), Alu.mult)
            nc.vector.tensor_tensor(t4, cur(m12), prv(m21), Alu.mult)
            nc.vector.tensor_tensor(t5, cur(m12), prv(m22), Alu.mult)
            nc.vector.tensor_tensor(t6, cur(m12), prv(q2), Alu.mult)
            nc.vector.tensor_tensor(cur(m11), t1, t4, Alu.add)
            nc.vector.tensor_tensor(cur(m12), t2, t5, Alu.add)
            nc.vector.tensor_tensor(t3, t3, t6, Alu.add)
            nc.vector.tensor_tensor(cur(q1), t3, cur(q1), Alu.add)
            nc.gpsimd.tensor_tensor(g1, cur(m21), prv(m11), Alu.mult)
            nc.gpsimd.tensor_tensor(g2, cur(m21), prv(m12), Alu.mult)
            nc.gpsimd.tensor_tensor(g3, cur(m21), prv(q1), Alu.mult)
            nc.gpsimd.tensor_tensor(g4, cur(m22), prv(m21), Alu.mult)
            nc.gpsimd.tensor_tensor(g5, cur(m22), prv(m22), Alu.mult)
            nc.gpsimd.tensor_tensor(g6, cur(m22), prv(q2), Alu.mult)
            nc.gpsimd.tensor_tensor(cur(m21), g1, g4, Alu.add)
            nc.gpsimd.tensor_tensor(cur(m22), g2, g5, Alu.add)
            nc.gpsimd.tensor_tensor(g3, g3, g6, Alu.add)
            nc.gpsimd.tensor_tensor(cur(q2), g3, cur(q2), Alu.add)

    with TileContext(nc) as tc, \
         (tc.For_i(0, loop_n, 1, name="rep") if loop_n else nullcontext()):
      with tc.tile_pool(name="xa", bufs=1) as xa_pool:
        t_x = xa_pool.tile([128, CPS, L1], fp32, name="xs")
        t_l8 = xa_pool.tile([ROWS, 5], fp32, name="l8")
        t_l8b = xa_pool.tile([ROWS, 3], bf16, name="l8b")
        t_fk = xa_pool.tile([ROWS, 3, 5], fp32, name="fk")
        t_fkb = xa_pool.tile([ROWS, 3, 3], bf16, name="fkb")
        dma(out=t_l8[:], in_=last8_in, eng=nc.gpsimd)
        dma(out=t_fk[:], in_=firstk_in, eng=nc.sync)
        nc.vector.tensor_copy(t_l8b[:], t_l8[:, 2:5])
        nc.vector.tensor_copy(t_fkb[:], t_fk[:, :, 2:5])

        def stage_tail(st, nplane):
            v = st[0, 0, :].copy()
            v.ap = bass_rust.VecI64Pair([[nplane * TP, ROWS], [TP, nplane], [1, 1]])
            v.offset = v.offset + PAD + T - 1
            return v

        dma(out=stage_tail(st_na, 2), in_=t_l8[:, 0:2], eng=nc.gpsimd)
        dma(out=stage_tail(st_b, 3), in_=t_l8b[:], eng=nc.sync)

        def stage_first(st, nplane, plane):
            v = st[0, 0, :].copy()
            v.ap = bass_rust.VecI64Pair([[nplane * TP, ROWS], [SEGLEN, 3]])
            v.offset = v.offset + PAD + plane * TP
            return v

        for p in range(2):
            dma(out=stage_first(st_na, 2, p), in_=t_fk[:, :, p],
                eng=nc.sync if p == 0 else nc.gpsimd)
        for p in range(3):
            dma(out=stage_first(st_b, 3, p), in_=t_fkb[:, :, p],
                eng=nc.sync if p % 2 == 0 else nc.gpsimd)
        xv = t_x[:].rearrange("p a b -> p (a b)")
        xs = x_in.rearrange("r (p s) -> (r p) s", p=NSTR)
        # column-split across the three DMA-capable queues: per-DMA cost in
        # the DGE model scales with per-row bytes, so three 1/3-width DMAs
        # on distinct queues land x in ~1/3 the time
        _xq = [nc.sync, nc.scalar]
        XW = STR // 6
        for i in range(6):
            lo = i * XW
            hi = STR if i == 5 else (i + 1) * XW
            dma(out=xv[:, lo:hi], in_=xs[:, lo:hi], eng=_xq[i % 2])
        # b-window tiles 0..6 persist here so their scatters can run during
        # phase B on otherwise-idle queues (tile 7 scatters in-loop).
        with tc.tile_pool(name="bwin", bufs=1) as bw_pool:
          t_bw = [bw_pool.tile([128, 3, WIN], bf16, name=f"bw_{ti}")
                  for ti in range(6)]
          # ---------------- phase A: coefficient generation ----------------
          with tc.tile_pool(name="segc", bufs=1) as sc_pool, \
               tc.tile_pool(name="segp", bufs=2) as sp_pool:
            t_iota = sc_pool.tile([128, WIN], fp32, name="iota_t")
            dma(out=t_iota[:], in_=iota_in, eng=nc.sync)
            t_v0a = sc_pool.tile([128, 8, 5], fp32, name="v0all")
            t_v1a = sc_pool.tile([128, 8, 5], fp32, name="v1all")
            t_w0a = sc_pool.tile([128, 8, 1], fp32, name="w0all")
            t_da = sc_pool.tile([128, 8, 5], fp32, name="dall")
            t_one = sc_pool.tile([128, 1], fp32, name="onec")
            dma(out=t_v0a[:], in_=v0_in, eng=nc.sync)
            dma(out=t_v1a[:], in_=v1_in, eng=nc.sync)
            dma(out=t_w0a[:], in_=w0_in, eng=nc.sync)
            nc.vector.memset(t_one[:], 1.0)
            nc.vector.tensor_tensor(t_da[:], t_v1a[:], t_v0a[:], Alu.subtract)
            # fold the per-window offset into per-partition scale/bias so the
            # interpolated-logit consumers read the SHARED iota tile directly:
            #   plane(t) = d*(DELTA*iota + w0) + v0 = (d*DELTA)*iota + (d*w0+v0)
            t_sca = sc_pool.tile([128, 8, 5], fp32, name="scall")
            t_bia = sc_pool.tile([128, 8, 5], fp32, name="biall")
            nc.vector.tensor_scalar_mul(t_sca[:], t_da[:], DELTA)
            nc.vector.tensor_tensor(
                t_bia[:], t_da[:],
                t_w0a[:].broadcast_to([128, 8, 5]), Alu.mult)
            nc.vector.tensor_tensor(t_bia[:], t_bia[:], t_v0a[:], Alu.add)
            _sc_pat = [nc.gpsimd, nc.sync, nc.gpsimd, nc.sync, nc.gpsimd,
                       nc.sync, nc.gpsimd, nc.gpsimd, nc.sync, nc.sync]
            _sc_i = [0]

            def _sc_next(_):
                e = _sc_pat[_sc_i[0] % len(_sc_pat)]
                _sc_i[0] += 1
                return e
            _late_b = []
            for ti in range(8):
                t_sc = t_sca[:, ti, :]
                t_bi = t_bia[:, ti, :]

                t_t1 = sp_pool.tile([128, WIN], fp32, name=f"t1_{ti}", tag="t1")
                t_t2 = sp_pool.tile([128, WIN], fp32, name=f"t2_{ti}", tag="t2")
                nc.scalar.activation(t_t1[:], t_iota[:], Act.Tanh,
                                     bias=t_bi[:, 0:1], scale=t_sc[:, 0:1])
                nc.scalar.activation(t_t2[:], t_iota[:], Act.Tanh,
                                     bias=t_bi[:, 1:2], scale=t_sc[:, 1:2])
                t_na = sp_pool.tile([128, 2, WIN], fp32, name=f"na_{ti}", tag="na")
                t_na1 = t_na[:, 0, :]
                nc.vector.tensor_scalar_mul(t_na1, t_t1[:], -2.0 * STAB)
                t_st2 = sp_pool.tile([128, WIN], fp32, name=f"st2_{ti}", tag="st2")
                nc.vector.tensor_scalar_mul(t_st2[:], t_t2[:], STAB)
                # vv = 1 - stab*t2 on ACT (from t2 directly)
                t_vv = sp_pool.tile([128, WIN], fp32, name=f"vv_{ti}", tag="vv")
                nc.vector.tensor_scalar(t_vv[:], t_st2[:], -1.0, 1.0,
                                        Alu.mult, Alu.add)
                t_u = sp_pool.tile([128, WIN], fp32, name=f"u_{ti}", tag="u")
                nc.scalar.activation(t_u[:], t_t1[:], Act.Abs)
                t_uv = sp_pool.tile([128, WIN], fp32, name=f"uv_{ti}", tag="uv")
                nc.gpsimd.tensor_tensor(t_uv[:], t_u[:], t_vv[:], Alu.mult)
                t_na2 = t_na[:, 1, :]
                nc.vector.scalar_tensor_tensor(t_na2, t_uv[:], -STAB, t_st2[:],
                                               Alu.mult, Alu.subtract)
                # FIR coefficient windows in bf16 (w*d + v0), one engine each
                if ti < 6:
                    t_b = t_bw[ti]
                else:
                    t_b = sp_pool.tile([128, 3, WIN], bf16, name=f"b_{ti}",
                                       tag="b")
                nc.vector.tensor_scalar(t_b[:, 0, :], t_iota[:], t_sc[:, 2:3],
                                        t_bi[:, 2:3], Alu.mult, Alu.add)
                nc.gpsimd.tensor_scalar(t_b[:, 1, :], t_iota[:], t_sc[:, 3:4],
                                        t_bi[:, 3:4], Alu.mult, Alu.add)
                nc.gpsimd.tensor_scalar(t_b[:, 2, :], t_iota[:], t_sc[:, 4:5],
                                        t_bi[:, 4:5], Alu.mult, Alu.add)
                scatter_plane(ti, t_na[:], na_dst, 2, _sc_next)
                if ti >= 6:
                    _late_b.append((ti, t_b))

            # tiles 6-7 b-scatters go last on ACT so they never delay the
            # tile-7 coefficient computes that gate the phase boundary
            for ti, t_b in _late_b:
                scatter_plane(ti, t_b[:], b_dst, 3, lambda i: nc.scalar)

          # merged coefficient gathers: one DMA per plane across all rows
          def plane_src(st, nplane, plane, dt=fp32):
              v = st[0, 0, :].copy()
              v.ap = bass_rust.VecI64Pair(
                  [[nplane * TP, ROWS], [STR, NSTR], [1, STR]])
              v.offset = v.offset + plane * TP + PAD
              return v

          # ---------------- phase B: chunked 3-solution scan ----------------
          # y_zero overwrites t_x in place (x[s] is dead once step s runs).
          with tc.tile_pool(name="hpool", bufs=1) as h_pool:
            t_h1 = h_pool.tile([128, CPS, L1], fp32, name="h1s")
            t_h2 = h_pool.tile([128, CPS, L1], fp32, name="h2s")
            t_ms = h_pool.tile([128, 2, CPS], fp32, name="mscr")
            t_m1 = t_ms[:, 0, :]
            t_m2g = t_ms[:, 1, :]
            with tc.tile_pool(name="apool", bufs=1) as a_pool:
              t_a1 = a_pool.tile([128, CPS, L1], fp32, name="a1s")
              t_a2 = a_pool.tile([128, CPS, L1], fp32, name="a2s")
              t_g = a_pool.tile([128, 2, CPS], fp32, name="gscr")
              t_g1 = t_g[:, 0, :]
              t_g2 = t_g[:, 1, :]
              def col_split_gather(dst2d, plane, queues, nch=6):
                  # split a [128, STR] gather into column chunks alternating
                  # across the HWDGE queues so the transfers spread over
                  # multiple DMA rings
                  w = STR // nch
                  for i in range(nch):
                      lo = i * w
                      hi = STR if i == nch - 1 else (i + 1) * w
                      src = st_na[0, 0, :].copy()
                      src.ap = bass_rust.VecI64Pair(
                          [[2 * TP, ROWS], [STR, NSTR], [1, hi - lo]])
                      src.offset = src.offset + plane * TP + PAD + lo
                      dma(out=dst2d[:, lo:hi], in_=src,
                          eng=queues[i % len(queues)])

              col_split_gather(t_a1[:].rearrange("p a b -> p (a b)"), 0,
                               [nc.sync, nc.scalar])
              col_split_gather(t_a2[:].rearrange("p a b -> p (a b)"), 1,
                               [nc.scalar, nc.sync])


              # deferred b-plane scatters: data has been ready since phase A,
              # so SP/ACT drain these under the phase-B scan, after the
              # coefficient gathers already in their queues.
              _bs_i = [0]

              def _bs_next(_):
                  _bs_i[0] += 1
                  return nc.sync if _bs_i[0] % 2 else nc.scalar
              for ti in range(6):
                  scatter_plane(ti, t_bw[ti][:], b_dst, 3, _bs_next)

              for s in range(L1):
                a1s = t_a1[:, :, s]
                a2s = t_a2[:, :, s]
                if s == 0:
                    # yz[0] = x[0] already in place
                    nc.vector.tensor_copy(t_h1[:, :, 0], a1s)
                    nc.gpsimd.tensor_copy(t_h2[:, :, 0], a2s)
                elif s == 1:
                    nc.vector.tensor_tensor(t_m1, a1s, t_x[:, :, 0], Alu.mult)
                    nc.vector.tensor_tensor(t_x[:, :, 1], t_x[:, :, 1], t_m1,
                                            Alu.add)
                    nc.vector.tensor_tensor(t_g1, a1s, t_h1[:, :, 0], Alu.mult)
                    nc.vector.tensor_tensor(t_h1[:, :, 1], t_g1, a2s, Alu.add)
                    nc.gpsimd.tensor_tensor(t_h2[:, :, 1], a1s, t_h2[:, :, 0],
                                            Alu.mult)
                else:
                    nc.gpsimd.tensor_tensor(t_m2g, a2s, t_x[:, :, s - 2],
                                            Alu.mult)
                    nc.gpsimd.tensor_tensor(t_x[:, :, s], t_x[:, :, s],
                                            t_m2g, Alu.add)
                    nc.vector.tensor_tensor(t_m1, a1s, t_x[:, :, s - 1], Alu.mult)
                    nc.vector.tensor_tensor(t_x[:, :, s], t_x[:, :, s], t_m1,
                                            Alu.add)
                    # h1 split: the 2-step-old product on GPSIMD (no stall)
                    nc.gpsimd.tensor_tensor(t_g2, a2s, t_h1[:, :, s - 2], Alu.mult)
                    nc.vector.tensor_tensor(t_g1, a1s, t_h1[:, :, s - 1], Alu.mult)
                    nc.vector.tensor_tensor(t_h1[:, :, s], t_g1, t_g2, Alu.add)
                    # h2 on GPSIMD
                    nc.gpsimd.tensor_tensor(t_m2g, a1s, t_h2[:, :, s - 1], Alu.mult)
                    nc.gpsimd.tensor_tensor(t_ms[:, 0, :], a2s, t_h2[:, :, s - 2],
                                            Alu.mult)
                    nc.gpsimd.tensor_tensor(t_h2[:, :, s], t_m2g, t_ms[:, 0, :],
                                            Alu.add)

            # ---- phase C: hierarchical combine (apool space now reusable) ----
            with tc.tile_pool(name="comb", bufs=1) as pool:
              BL, LB = 16, 16
              lad = [pool.tile([128, BL, LB], fp32, name=f"lad{i}")
                     for i in range(6)]
              t_al = pool.tile([128, BL, LB], fp32, name="alph")
              t_be = pool.tile([128, BL, LB], fp32, name="beta")
              t_p1 = t_al[:].rearrange("p a b -> p (a b)")
              t_p2 = t_be[:].rearrange("p a b -> p (a b)")
              srcs = [t_h1[:, :, L1 - 1], t_h2[:, :, L1 - 1],
                      t_h1[:, :, L1 - 2], t_h2[:, :, L1 - 2],
                      t_x[:, :, L1 - 1], t_x[:, :, L1 - 2]]
              for i in range(6):
                eng = nc.vector if i % 2 == 0 else nc.gpsimd
                eng.tensor_copy(lad[i][:].rearrange("p a b -> p (a b)"),
                                srcs[i])
              tmps = [t_p1[:, 0:BL], t_p1[:, BL:2 * BL], t_p1[:, 2 * BL:3 * BL],
                      t_p2[:, 0:BL], t_p2[:, BL:2 * BL], t_p2[:, 2 * BL:3 * BL]]
              gtmps = [t_p1[:, 4 * BL:5 * BL], t_p1[:, 5 * BL:6 * BL],
                       t_p1[:, 6 * BL:7 * BL], t_p2[:, 4 * BL:5 * BL],
                       t_p2[:, 5 * BL:6 * BL], t_p2[:, 6 * BL:7 * BL]]
              prefix_chain([a[:] for a in lad], tmps, 128, BL, LB, gp_tmps=gtmps)

              blk = [pool.tile([128, 1, BL], fp32, name=f"blk{i}")
                     for i in range(6)]
              for i in range(6):
                nc.vector.tensor_copy(blk[i][:, 0, :], lad[i][:, :, LB - 1])
              btmp = [t_p1[:, 3 * BL + i:3 * BL + i + 1] for i in range(6)]
              gbtmp = [t_p2[:, 3 * BL + i:3 * BL + i + 1] for i in range(6)]
              prefix_chain([a[:] for a in blk], btmp, 128, 1, BL, gp_tmps=gbtmp)

              t_cmp = pool.tile([128, 6], fp32, name="cmp")
              for i in range(6):
                nc.vector.tensor_copy(t_cmp[:, i:i + 1], blk[i][:, 0, BL - 1:BL])
              dma(out=st_cmp, in_=t_cmp[:], eng=nc.sync)

              t_row = pool.tile([4, NSTR, 6], fp32, name="rowc")
              dma(out=t_row[:], in_=st_cmp.rearrange("(r j) c -> r j c", r=ROWS),
                  eng=nc.sync)
              rcomp = [t_row[:, :, i].rearrange("r (b l) -> r b l", b=4)
                       for i in range(6)]
              rtmp = [pool.tile([4, 4], fp32, name=f"rtmp{i}") for i in range(6)]
              rgtmp = [pool.tile([4, 4], fp32, name=f"rgtmp{i}") for i in range(6)]
              prefix_chain(rcomp, [a[:] for a in rtmp], 4, 4, 8,
                           gp_tmps=[a[:] for a in rgtmp])
              rblk = [pool.tile([4, 1, 4], fp32, name=f"rblk{i}")
                      for i in range(6)]
              for i in range(6):
                nc.vector.tensor_copy(rblk[i][:, 0, :], rcomp[i][:, :, 7])
              rbt = [pool.tile([4, 1], fp32, name=f"rbt{i}") for i in range(6)]
              rgbt = [pool.tile([4, 1], fp32, name=f"rgbt{i}") for i in range(6)]
              prefix_chain([a[:] for a in rblk], [a[:] for a in rbt], 4, 1, 4,
                           gp_tmps=[a[:] for a in rgbt])

              qb1 = pool.tile([4, 4], fp32, name="qb1")
              qb2 = pool.tile([4, 4], fp32, name="qb2")
              nc.vector.memset(qb1[:, 0:1], 0.0)
              nc.vector.memset(qb2[:, 0:1], 0.0)
              nc.vector.tensor_copy(qb1[:, 1:4], rblk[4][:, 0, 0:3])
              nc.vector.tensor_copy(qb2[:, 1:4], rblk[5][:, 0, 0:3])
              sincl1 = pool.tile([4, 4, 8], fp32, name="sincl1")
              sincl2 = pool.tile([4, 4, 8], fp32, name="sincl2")
              tq1 = pool.tile([4, 4, 8], fp32, name="tq1")
              qb1b = qb1[:].unsqueeze(-1).broadcast_to([4, 4, 8])
              qb2b = qb2[:].unsqueeze(-1).broadcast_to([4, 4, 8])
              nc.vector.tensor_tensor(sincl1[:], rcomp[0], qb1b, Alu.mult)
              nc.vector.tensor_tensor(tq1[:], rcomp[1], qb2b, Alu.mult)
              nc.vector.tensor_tensor(sincl1[:], sincl1[:], tq1[:], Alu.add)
              nc.vector.tensor_tensor(sincl1[:], sincl1[:], rcomp[4], Alu.add)
              nc.vector.tensor_tensor(sincl2[:], rcomp[2], qb1b, Alu.mult)
              nc.vector.tensor_tensor(tq1[:], rcomp[3], qb2b, Alu.mult)
              nc.vector.tensor_tensor(sincl2[:], sincl2[:], tq1[:], Alu.add)
              nc.vector.tensor_tensor(sincl2[:], sincl2[:], rcomp[5], Alu.add)
              sent = pool.tile([4, NSTR, 2], fp32, name="sent")
              nc.vector.memset(sent[:, 0, :], 0.0)
              si1 = sincl1[:].rearrange("r b l -> r (b l)")
              si2 = sincl2[:].rearrange("r b l -> r (b l)")
              nc.vector.tensor_copy(sent[:, 1:NSTR, 0], si1[:, 0:NSTR - 1])
              nc.vector.tensor_copy(sent[:, 1:NSTR, 1], si2[:, 0:NSTR - 1])
              dma(out=st_sin, in_=sent[:], eng=nc.sync)

              t_sstr = pool.tile([128, 2], fp32, name="sstr")
              dma(out=t_sstr[:], in_=st_sin.rearrange("r j c -> (r j) c"),
                  eng=nc.sync)

              sb1 = pool.tile([128, BL], fp32, name="sb1")
              sb2 = pool.tile([128, BL], fp32, name="sb2")
              s1 = t_sstr[:, 0:1]
              s2 = t_sstr[:, 1:2]
              nc.vector.tensor_copy(sb1[:, 0:1], t_sstr[:, 0:1])
              nc.gpsimd.tensor_copy(sb2[:, 0:1], t_sstr[:, 1:2])
              tb = pool.tile([128, BL - 1], fp32, name="tb")
              nc.vector.tensor_scalar_mul(tb[:], blk[0][:, 0, 0:BL - 1], s1)
              nc.vector.scalar_tensor_tensor(sb1[:, 1:BL], blk[1][:, 0, 0:BL - 1],
                                             s2, tb[:], Alu.mult, Alu.add)
              nc.vector.tensor_tensor(sb1[:, 1:BL], sb1[:, 1:BL],
                                      blk[4][:, 0, 0:BL - 1], Alu.add)
              tb2 = pool.tile([128, BL - 1], fp32, name="tb2")
              nc.vector.tensor_scalar_mul(tb2[:], blk[2][:, 0, 0:BL - 1], s1)
              nc.vector.scalar_tensor_tensor(sb2[:, 1:BL], blk[3][:, 0, 0:BL - 1],
                                             s2, tb2[:], Alu.mult, Alu.add)
              nc.gpsimd.tensor_tensor(sb2[:, 1:BL], sb2[:, 1:BL],
                                      blk[5][:, 0, 0:BL - 1], Alu.add)

              nc.vector.tensor_copy(t_al[:, :, 0], sb1[:])
              nc.gpsimd.tensor_copy(t_be[:, :, 0], sb2[:])
              sb1b = sb1[:].unsqueeze(-1).broadcast_to([128, BL, LB - 1])
              sb2b = sb2[:].unsqueeze(-1).broadcast_to([128, BL, LB - 1])
              tq = pool.tile([128, BL, LB - 1], fp32, name="tqs")
              nc.vector.tensor_tensor(t_al[:, :, 1:LB], lad[0][:, :, 0:LB - 1],
                                      sb1b, Alu.mult)
              nc.vector.tensor_tensor(tq[:], lad[1][:, :, 0:LB - 1], sb2b,
                                      Alu.mult)
              nc.vector.tensor_tensor(t_al[:, :, 1:LB], t_al[:, :, 1:LB], tq[:],
                                      Alu.add)
              nc.vector.tensor_tensor(t_al[:, :, 1:LB], t_al[:, :, 1:LB],
                                      lad[4][:, :, 0:LB - 1], Alu.add)
              tq2 = pool.tile([128, BL, LB - 1], fp32, name="tqs2")
              nc.gpsimd.tensor_tensor(t_be[:, :, 1:LB], lad[2][:, :, 0:LB - 1],
                                      sb1b, Alu.mult)
              nc.gpsimd.tensor_tensor(tq2[:], lad[3][:, :, 0:LB - 1], sb2b,
                                      Alu.mult)
              nc.gpsimd.tensor_tensor(t_be[:, :, 1:LB], t_be[:, :, 1:LB], tq2[:],
                                      Alu.add)
              nc.gpsimd.tensor_tensor(t_be[:, :, 1:LB], t_be[:, :, 1:LB],
                                      lad[5][:, :, 0:LB - 1], Alu.add)

              # ---- correction (in place): y = y_zero + alpha*h1 + beta*h2 ----
              alv = t_al[:].rearrange("p a b -> p (a b)")  # [128, 256]
              bev = t_be[:].rearrange("p a b -> p (a b)")
              alb = alv.unsqueeze(-1).broadcast_to([128, CPS, L1])
              beb = bev.unsqueeze(-1).broadcast_to([128, CPS, L1])
              C0 = 110
              nc.vector.tensor_tensor(t_h1[:, 0:C0], t_h1[:, 0:C0], alb[:, 0:C0],
                                      Alu.mult)
              nc.vector.tensor_tensor(t_x[:, 0:C0], t_x[:, 0:C0], t_h1[:, 0:C0],
                                      Alu.add)
              nc.vector.tensor_tensor(t_h2[:, 0:C0], t_h2[:, 0:C0], beb[:, 0:C0],
                                      Alu.mult)
              nc.vector.tensor_tensor(t_x[:, 0:C0], t_x[:, 0:C0], t_h2[:, 0:C0],
                                      Alu.add)
              nc.gpsimd.tensor_tensor(t_h1[:, C0:CPS], t_h1[:, C0:CPS],
                                      alb[:, C0:CPS], Alu.mult)
              nc.gpsimd.tensor_tensor(t_x[:, C0:CPS], t_x[:, C0:CPS],
                                      t_h1[:, C0:CPS], Alu.add)
              nc.gpsimd.tensor_tensor(t_h2[:, C0:CPS], t_h2[:, C0:CPS],
                                      beb[:, C0:CPS], Alu.mult)
              nc.gpsimd.tensor_tensor(t_x[:, C0:CPS], t_x[:, C0:CPS],
                                      t_h2[:, C0:CPS], Alu.add)

        # ---------------- phase D: FIR in scan layout ----------------
        # t_x now holds the full IIR output y in [128, 8192] time-linear
        # layout (partition = (row, stretch)).
        with tc.tile_pool(name="firp", bufs=1) as fp_pool:
            t_b0 = fp_pool.tile([128, STR], bf16, name="b0s")
            t_b1 = fp_pool.tile([128, STR], bf16, name="b1s")
            t_b2 = fp_pool.tile([128, STR], bf16, name="b2s")
            def b_split_gather(dst2d, plane, queues, nch=4):
                w = STR // nch
                for i in range(nch):
                    lo = i * w
                    src = st_b[0, 0, :].copy()
                    src.ap = bass_rust.VecI64Pair(
                        [[3 * TP, ROWS], [STR, NSTR], [1, w]])
                    src.offset = src.offset + plane * TP + PAD + lo
                    dma(out=dst2d[:, lo:lo + w], in_=src,
                        eng=queues[i % len(queues)])

            b_split_gather(t_b0[:], 0, [nc.sync, nc.scalar])
            b_split_gather(t_b1[:], 1, [nc.scalar, nc.sync])
            b_split_gather(t_b2[:], 2, [nc.sync, nc.scalar])
            # 2-sample halo from the previous partition (zero at row starts)
            t_halo = fp_pool.tile([128, 2], fp32, name="halo")
            nc.vector.memset(t_halo[:], 0.0)
            yv = t_x[:].rearrange("p a b -> p (a b)")  # [128, 8192]
            dma(out=t_halo[1:128, :], in_=yv[0:127, STR - 2:STR], eng=nc.sync)
            for p in (32, 64, 96):
                nc.vector.memset(t_halo[p:p + 1, :], 0.0)
            # y in bf16 for 2x products (half-split cast unblocks the FIR)
            t_yb = fp_pool.tile([128, STR + 2], bf16, name="ybf")
            HC = STR // 2
            nc.scalar.activation(t_yb[:, 2:2 + HC], yv[:, 0:HC], Act.Copy)
            nc.scalar.activation(t_yb[:, 0:2], t_halo[:], Act.Copy)
            nc.scalar.activation(t_yb[:, 2 + HC:], yv[:, HC:], Act.Copy)
            t_o = fp_pool.tile([128, STR], bf16, name="obf")
            t_f = fp_pool.tile([128, STR], bf16, name="fbf")
            t_f2 = fp_pool.tile([128, STR], bf16, name="f2bf")
            t_of = fp_pool.tile([128, STR], fp32, name="ofs")
            # products split in halves, pipelined with the yb cast halves;
            # b2 product on GPSIMD in parallel with the DVE products
            HP = STR // 2
            for h in range(2):
                lo, hi = h * HP, (h + 1) * HP
                nc.gpsimd.tensor_tensor(t_f2[:, lo:hi], t_b2[:, lo:hi],
                                        t_yb[:, lo:hi], Alu.mult)
                nc.vector.tensor_tensor(t_o[:, lo:hi], t_b0[:, lo:hi],
                                        t_yb[:, 2 + lo:2 + hi], Alu.mult)
                nc.vector.tensor_tensor(t_f[:, lo:hi], t_b1[:, lo:hi],
                                        t_yb[:, 1 + lo:1 + hi], Alu.mult)
                nc.vector.tensor_tensor(t_o[:, lo:hi], t_o[:, lo:hi],
                                        t_f[:, lo:hi], Alu.add)
            QQ = STR // 4
            yo = y_out.rearrange("r (p s) -> (r p) s", p=NSTR)
            _qeng = [nc.vector, nc.gpsimd, nc.vector, nc.gpsimd]
            _qdma = [nc.sync, nc.scalar]
            for q in range(4):
                lo, hi = q * QQ, (q + 1) * QQ
                _qeng[q].tensor_tensor(t_of[:, lo:hi], t_o[:, lo:hi],
                                       t_f2[:, lo:hi], Alu.add)
                for j in range(2):
                    w = QQ // 2
                    dma(out=yo[:, lo + j * w:lo + (j + 1) * w],
                        in_=t_of[:, lo + j * w:lo + (j + 1) * w],
                        eng=_qdma[(2 * q + j) % 2])

    _fix_multi_waits(nc)
    return nc


_NC_CACHE = None


def _time_marginal(in_maps, n1=1, n2=49, reps=8):
    """(T(loop n2) - T(loop n1)) / (n2 - n1) with device-resident operands."""
    import time, jax
    from jax.sharding import Mesh, NamedSharding, PartitionSpec
    from concourse import bass2jax

    walls = {}
    for n in (n1, n2):
        nc_n = build_program(loop_n=n)
        cap = {}
        orig_jit = jax.jit

        def capturing_jit(f, **kw):
            j = orig_jit(f, **kw)

            def wrapper(*a, **k):
                cap['fn'], cap['args'] = j, a
                return j(*a, **k)
            return wrapper

        jax.jit = capturing_jit
        try:
            bass2jax.run_bass_via_pjrt(nc_n, in_maps, n_cores=NCORES)
        finally:
            jax.jit = orig_jit
        fn, args = cap['fn'], cap['args']
        devs = jax.devices()[:NCORES]
        mesh = Mesh(np.asarray(devs), ("core",))
        sh = NamedSharding(mesh, PartitionSpec("core"))
        in_args = [jax.device_put(a, sh) for a in args[:-1]]
        zsets = [jax.device_put(np.asarray(args[-1]), sh)
                 for _ in range(reps + 1)]
        jax.block_until_ready((in_args, zsets))
        jax.block_until_ready(fn(*in_args, zsets[0]))  # warm
        best = None
        for r in range(1, reps + 1):
            t0 = time.perf_counter()
            jax.block_until_ready(fn(*in_args, zsets[r]))
            w = time.perf_counter() - t0
            best = w if best is None else min(best, w)
        walls[n] = best
        print(f"  loop{n} wall: {best*1e3:.3f} ms")
    d = (walls[n2] - walls[n1]) / (n2 - n1)
    print(f"  marginal per-iteration: {d*1e6:.1f} us")
    return int(d * 1e9)


def core_inputs(x, cl, c):
    """Per-core input map (host tables + row slice) for core c."""
    w0, iota = host_tables()
    rows = slice(c * ROWS, (c + 1) * ROWS)
    v0, v1 = host_v0v1(np.ascontiguousarray(np.asarray(cl, np.float32))[rows])
    v = np.asarray(cl, np.float64)[rows][:, 255, :]  # logits at t = T-1
    stab = 1.0 - 1e-3
    a1 = 2.0 * np.tanh(v[:, 0]) * stab
    a2 = 0.5 * ((2.0 - np.abs(a1)) * np.tanh(v[:, 1]) * stab + np.abs(a1))
    last8 = np.stack([-a1, -a2, v[:, 2], v[:, 3], v[:, 4]],
                     axis=1).astype(np.float32)
    # super-block start samples t = SEGLEN*k sit exactly on knots 85k
    vk = np.asarray(cl, np.float64)[rows][:, [0, 85, 170], :]  # [4, 3, 5]
    a1k = 2.0 * np.tanh(vk[..., 0]) * stab
    a2k = 0.5 * ((2.0 - np.abs(a1k)) * np.tanh(vk[..., 1]) * stab
                 + np.abs(a1k))
    firstk = np.stack([-a1k, -a2k, vk[..., 2], vk[..., 3], vk[..., 4]],
                      axis=2).astype(np.float32)
    return {
        "x": np.ascontiguousarray(np.asarray(x, np.float32))[rows].copy(),
        "v0": v0, "v1": v1, "w0": w0, "iota": iota, "last8": last8,
        "firstk": firstk,
    }


def kernel(x, coeff_logits):
    """Full inputs -> full output, running the Bass kernel on 8 NeuronCores."""
    global _NC_CACHE
    _patch_tile_drain()
    from concourse.bass_utils import run_bass_kernel_spmd

    x = np.ascontiguousarray(np.asarray(x, dtype=np.float32))
    cl = np.ascontiguousarray(np.asarray(coeff_logits, dtype=np.float32))
    if _NC_CACHE is None:
        _NC_CACHE = build_program()
    nc = _NC_CACHE

    in_maps = [core_inputs(x, cl, c) for c in range(NCORES)]
    import os, time, jax
    do_time = bool(int(os.environ.get("KERNEL_TIME", "0")))
    global LAST_EXEC_NS
    res = run_bass_kernel_spmd(nc, in_maps, list(range(NCORES)))
    if do_time:
        try:
            # Marginal per-iteration device time: compile the same kernel
            # body wrapped in an on-device hardware loop (tc.For_i) with two
            # trip counts, run both NEFFs with all operands device-resident,
            # and report (T(n2) - T(n1)) / (n2 - n1).  The difference
            # cancels the axon-tunnel dispatch round-trip and any fixed NEFF
            # launch overhead shared by both programs, leaving the honest
            # steady-state hardware execution time of one kernel iteration.
            LAST_EXEC_NS = _time_marginal(in_maps)
        except Exception as e:
            import traceback
            traceback.print_exc()
            print("timing failed:", e)
            LAST_EXEC_NS = -1
    else:
        res = run_bass_kernel_spmd(nc, in_maps, list(range(NCORES)))
    out = np.empty((B, T), np.float32)
    for c in range(NCORES):
        out[c * ROWS:(c + 1) * ROWS] = res.results[c]["y"]
    return out


LAST_EXEC_NS = None

